# revision 1
# baseline (speedup 1.0000x reference)
"""Quantized (4-bit) LoRA linear for Trainium2, SPMD over 8 NeuronCores.

Math:  y[t,o] = sum_i x[t,i]*W[o,i] + bias[o] + 2.0 * sum_r (x@A^T)[t,r]*B[o,r]
where  W[o,i] = (nib[o,i] - zero[i]) * scale[i],  nib = unpacked 4-bit ints.

Rewrite with xs[t,i] = x[t,i]*scale[i]:
  y[t,o] = sum_i xs[t,i]*nib[o,i]        (PE matmul: fp16 xs x fp8 nib, both exact
                                          enough: nib in {0..15} is exact in fp8e4)
         + G[t,:] @ H[o,:]^T             (K=9 matmul folding LoRA + zero-correction)
         + bias[o]                       (fused into the DVE PSUM evacuation)
  G cols: 0-7 = u[t,r] = sum_i xs[t,i]*(A[r,i]/scale[i]) = (x @ A^T)[t,r],
          8   = c[t]   = sum_i xs[t,i]*zero[i]
  H rows: 0-7 = 2.0*B^T, 8 = -1

Sharding: 8-way token split (1024 tokens per core), each core computes the
full 4096 outs in two o-half passes. xs (64 KB/part) stays resident in SBUF
across both passes; the fp8 nib half (64 KB/part) streams through a shared
pool, reloaded for pass 1 pipelined behind pass 0's tail. u/G computed once
(pass 0) and reused in pass 1. All accumulation in PSUM (7+1 banks).
"""

import numpy as np

B, S, I, O = 4, 2048, 4096, 4096
T = B * S            # 8192 tokens
NCORES = 8
TC = T // NCORES     # 1024 tokens per core
OH = O // 2          # 2048 outs per pass
KC = I // 128        # 32 contraction chunks
TG = 4               # token tiles per token-group (512 tokens)
NG = TC // (TG * 128)  # 2 token groups per core

_CACHE = {}


def _build_program():
    import concourse.bacc as bacc
    import concourse.mybir as mybir
    import concourse.tile as tile

    fp16 = mybir.dt.float16
    fp32 = mybir.dt.float32
    fp8 = mybir.dt.float8e4

    nc = bacc.Bacc("TRN2", target_bir_lowering=False, debug=False)
    xsT = nc.dram_tensor("xsT", [I, TC], fp16, kind="ExternalInput")
    nibT = nc.dram_tensor("nibT", [I, O], fp8, kind="ExternalInput")
    aextT = nc.dram_tensor("aextT", [I, 9], fp16, kind="ExternalInput")
    hmat = nc.dram_tensor("hmat", [9, O], fp16, kind="ExternalInput")
    bias_bc = nc.dram_tensor("bias_bc", [128, O], fp32, kind="ExternalInput")
    y = nc.dram_tensor("y", [TC, O], fp32, kind="ExternalOutput")

    with tile.TileContext(nc) as tc:
        with (
            tc.tile_pool(name="nib", bufs=KC) as nib_pool,
            tc.tile_pool(name="consts", bufs=1) as const_pool,
            tc.tile_pool(name="xs", bufs=1) as xs_pool,
            tc.tile_pool(name="g", bufs=1) as g_pool,
            tc.tile_pool(name="out", bufs=3) as out_pool,
            tc.tile_pool(name="psum", bufs=8, space="PSUM") as psum_pool,
        ):
            h_tile = const_pool.tile([9, O], fp16, tag="h")
            bias_tile = const_pool.tile([128, O], fp32, tag="bias")
            aext_tiles = [None] * KC
            # xs resident across both passes: unique tag per tile, bufs=1
            xs_tiles = {}   # (tg, k) -> tile
            gts = [None] * NG

            for h in range(2):  # o-half pass
                o0 = h * OH
                nib_tiles = [None] * KC
                for tg in range(NG):
                    t0 = tg * TG * 128
                    # Interleave DMAs chunk-by-chunk with compute emission so
                    # the PE streams right behind the DMA (pass 0 tg 0 and the
                    # pass-1 nib reload both pipeline this way).
                    for k in range(KC):
                        if tg == 0:
                            nt = nib_pool.tile([128, OH], fp8, tag="nib",
                                               name=f"nib{h}_{k}")
                            nc.sync.dma_start(
                                nt[:], nibT[k * 128:(k + 1) * 128, o0:o0 + OH]
                            )
                            nib_tiles[k] = nt
                        if h == 0:
                            xt = xs_pool.tile([128, TG * 128], fp16,
                                              tag=f"xs{tg}_{k}", bufs=1,
                                              name=f"xs{tg}_{k}")
                            nc.sync.dma_start(
                                xt[:], xsT[k * 128:(k + 1) * 128,
                                           t0:t0 + TG * 128]
                            )
                            xs_tiles[(tg, k)] = xt
                            if tg == 0:
                                at = const_pool.tile([128, 9], fp16,
                                                     tag=f"aext{k}",
                                                     name=f"aext{k}")
                                nc.sync.dma_start(
                                    at[:], aextT[k * 128:(k + 1) * 128, :]
                                )
                                aext_tiles[k] = at
                    if h == 0 and tg == 0:
                        nc.sync.dma_start(h_tile[:], hmat[:, :])
                        nc.sync.dma_start(bias_tile[:], bias_bc[:, :])

                    # tt0's main matmuls are interleaved per-chunk with the
                    # u matmuls (pass 0) so the PE starts after chunk 0 lands.
                    ps0 = [
                        psum_pool.tile([128, 512], fp32, tag="mm",
                                       name=f"mm{h}_{tg}_0_{j}")
                        for j in range(4)
                    ]
                    if h == 0:
                        # up shares the mm pool slots (it is released by the
                        # gt copy before the 8th concurrent mm bank is needed)
                        up = psum_pool.tile([9, TG * 128], fp32, tag="mm",
                                            name=f"up{tg}")
                    for k in range(KC):
                        if h == 0:
                            nc.tensor.matmul(
                                up[:], aext_tiles[k][:], xs_tiles[(tg, k)][:],
                                start=(k == 0), stop=(k == KC - 1),
                            )
                        lhsT = xs_tiles[(tg, k)][:, 0:128]
                        for j in range(4):
                            nc.tensor.matmul(
                                ps0[j][:], lhsT,
                                nib_tiles[k][:, j * 512:(j + 1) * 512],
                                start=(k == 0), stop=False,
                            )
                    if h == 0:
                        gt = g_pool.tile([9, TG * 128], fp16, tag=f"g{tg}",
                                         bufs=1, name=f"g{tg}")
                        nc.vector.tensor_copy(gt[:, :], up[:])
                        gts[tg] = gt
                    gt = gts[tg]

                    for tt in range(TG):
                        if tt == 0:
                            ps = ps0
                        else:
                            ps = [
                                psum_pool.tile([128, 512], fp32, tag="mm",
                                               name=f"mm{h}_{tg}_{tt}_{j}")
                                for j in range(4)
                            ]
                            for k in range(KC):
                                lhsT = xs_tiles[(tg, k)][:,
                                                         tt * 128:(tt + 1) * 128]
                                for j in range(4):
                                    nc.tensor.matmul(
                                        ps[j][:], lhsT,
                                        nib_tiles[k][:, j * 512:(j + 1) * 512],
                                        start=(k == 0), stop=False,
                                    )
                        gs = gt[:, tt * 128:(tt + 1) * 128]
                        for j in range(4):
                            nc.tensor.matmul(
                                ps[j][:], gs,
                                h_tile[:, o0 + j * 512:o0 + (j + 1) * 512],
                                start=False, stop=True,
                            )
                        ot = out_pool.tile([128, OH], fp32, tag="out")
                        for j in range(4):
                            nc.vector.tensor_add(
                                ot[:, j * 512:(j + 1) * 512], ps[j][:],
                                bias_tile[:, o0 + j * 512:o0 + (j + 1) * 512],
                            )
                        trow = t0 + tt * 128
                        nc.sync.dma_start(y[trow:trow + 128, o0:o0 + OH], ot[:])
    nc.compile()
    return nc


def _prep_inputs(x, weight_quant, scale, zero, lora_A, lora_B, bias):
    """Host-side layout prep + sharding. Returns in_maps for 8 cores."""
    import ml_dtypes

    xs = (x.reshape(T, I).astype(np.float32) * scale[None, :]).astype(np.float16)
    xsT = np.ascontiguousarray(xs.T)  # [I, T]

    wq = weight_quant.astype(np.uint8)            # low byte only is populated
    nib = np.empty((O, I), np.uint8)
    nib[:, 0::2] = wq & 15
    nib[:, 1::2] = wq >> 4
    nibT = np.ascontiguousarray(nib.T.astype(ml_dtypes.float8_e4m3fn))  # [I, O]

    aextT = np.empty((I, 9), np.float16)
    aextT[:, 0:8] = (lora_A.astype(np.float32) / scale[None, :]).T
    aextT[:, 8] = zero
    aextT = np.ascontiguousarray(aextT)

    hmat = np.empty((9, O), np.float16)
    hmat[0:8, :] = 2.0 * lora_B.T
    hmat[8, :] = -1.0
    hmat = np.ascontiguousarray(hmat)
    bias_bc = np.ascontiguousarray(
        np.broadcast_to(bias.astype(np.float32), (128, O))
    )

    in_maps = []
    for c in range(NCORES):
        in_maps.append({
            "xsT": np.ascontiguousarray(xsT[:, c * TC:(c + 1) * TC]),
            "nibT": nibT,
            "aextT": aextT,
            "hmat": hmat,
            "bias_bc": bias_bc,
        })
    return in_maps


def run_on_cores(in_maps, trace=False):
    from concourse.bass_utils import run_bass_kernel_spmd

    if "nc" not in _CACHE:
        _CACHE["nc"] = _build_program()
    return run_bass_kernel_spmd(
        _CACHE["nc"], in_maps, list(range(NCORES)), trace=trace
    )


def kernel(x, weight_quant, scale, zero, lora_A, lora_B, bias):
    x = np.asarray(x)
    weight_quant = np.asarray(weight_quant)
    scale = np.asarray(scale, np.float32)
    zero = np.asarray(zero, np.float32)
    lora_A = np.asarray(lora_A, np.float32)
    lora_B = np.asarray(lora_B, np.float32)
    bias = np.asarray(bias, np.float32)

    in_maps = _prep_inputs(x, weight_quant, scale, zero, lora_A, lora_B, bias)
    res = run_on_cores(in_maps).results

    out = np.concatenate([res[c]["y"] for c in range(NCORES)], axis=0)
    return np.ascontiguousarray(out).reshape(B, S, O)



# revision 2
# speedup vs baseline: 3.5641x; 3.5641x over previous
"""Quantized (4-bit) LoRA linear for Trainium2, SPMD over 8 NeuronCores.

Math:  y[t,o] = sum_i x[t,i]*W[o,i] + bias[o] + 2.0 * sum_r (x@A^T)[t,r]*B[o,r]
where  W[o,i] = (nib[o,i] - zero[i]) * scale[i],  nib = unpacked 4-bit ints.

fp8 DoubleRow formulation (PE runs fp8e4m3 x fp8e4m3 in DoubleRow perf mode:
one instruction contracts K=256 at 0.5 cycles/row -> 4x the fp16 matmul
throughput of the cost model; verified on hw that subnormal fp8 inputs are
honored, not flushed):

  y[t,o] = sum_i (16*xs[t,i])_fp8 * ((nib[o,i]-7.5)/16)_fp8     main matmul
         + sum_e G[t,e]*H[e,o]                                  ext matmul
  with xs = x*scale.  (nib-7.5)/16 is EXACT in fp8e4m3 (4-bit significands,
  subnormal half-integers included), so the only main-path error is the fp8
  rounding of 16*xs: rel err ~1.79e-2 < 2e-2 gate (measured vs reference).

  Ext rows (host-computed, fp8):  G rows 0-7 = u_r/32 (u = x@A^T), H rows
  0-7 = 64*B^T (folds the 2.0 LoRA scaling); row 8 = fp8(16*zc) with H=1/16
  where zc[t] = sum_i xs[t,i]*(7.5-zero[i]) (zero-point correction); row 9 =
  fp8(-e1) with H=1/16 (e1 = fp8 residual of row 8, second-order exact);
  row 10 = ones with H=fp8(bias); row 11 = zero pad.  12 rows = [6,2]
  DoubleRow layout.

Sharding: 8-way token split (1024 tokens/core), each core computes all 4096
outs in 4 o-quarter passes (1024 wide).  xq (2KB/part per k2-chunk) resident;
nib streams through a 32-buf pool (one o-quarter = 16 chunks live, next
quarter prefetches).  Per psum group: 16 main + 1 ext DoubleRow matmuls into
one PSUM bank, DVE-evacuated to fp16, DMA'd out.  Output returned fp32.
"""

import numpy as np

B, S, I, O = 4, 2048, 4096, 4096
T = B * S            # 8192 tokens
NCORES = 8
TC = T // NCORES     # 1024 tokens per core
K2 = I // 256        # 16 DoubleRow contraction chunks
NQ = 4               # o-quarter passes
OQ = O // NQ         # 1024 outs per pass
NTT = TC // 128      # 8 token tiles per core

_CACHE = {}


def _build_program():
    import concourse.bacc as bacc
    import concourse.mybir as mybir
    import concourse.tile as tile

    fp16 = mybir.dt.float16
    fp32 = mybir.dt.float32
    fp8 = mybir.dt.float8e4
    DR = mybir.MatmulPerfMode.DoubleRow

    nc = bacc.Bacc("TRN2", target_bir_lowering=False, debug=False)
    xqH = nc.dram_tensor("xqH", [128, K2, 2, TC], fp8, kind="ExternalInput")
    nibH = nc.dram_tensor("nibH", [I // 2, 2, O], fp8, kind="ExternalInput")
    extG = nc.dram_tensor("extG", [6, 2, TC], fp8, kind="ExternalInput")
    extH = nc.dram_tensor("extH", [6, 2, O], fp8, kind="ExternalInput")
    y = nc.dram_tensor("y", [TC, O], fp16, kind="ExternalOutput")

    with tile.TileContext(nc) as tc:
        with (
            tc.tile_pool(name="xq", bufs=1) as xq_pool,
            tc.tile_pool(name="nib", bufs=32) as nib_pool,
            tc.tile_pool(name="ext", bufs=1) as ext_pool,
            tc.tile_pool(name="out", bufs=3) as out_pool,
            tc.tile_pool(name="psum", bufs=8, space="PSUM") as psum_pool,
        ):
            eg = ext_pool.tile([6, 2, TC], fp8, tag="eg")
            eh = ext_pool.tile([6, 2, O], fp8, tag="eh")
            nc.sync.dma_start(eg[:], extG[:, :, :])
            nc.sync.dma_start(eh[:], extH[:, :, :])

            xq_tiles = [None] * K2
            for q in range(NQ):
                o0 = q * OQ
                nib_tiles = [None] * K2
                for k2 in range(K2):
                    nt = nib_pool.tile([128, 2, OQ], fp8, tag="nib",
                                       name=f"nib{q}_{k2}")
                    nc.sync.dma_start(
                        nt[:], nibH[k2 * 128:(k2 + 1) * 128, :, o0:o0 + OQ]
                    )
                    nib_tiles[k2] = nt
                    if q == 0:
                        xt = xq_pool.tile([128, 2, TC], fp8, tag=f"xq{k2}",
                                          name=f"xq{k2}")
                        nc.sync.dma_start(xt[:], xqH[:, k2, :, :])
                        xq_tiles[k2] = xt

                for tt in range(NTT):
                    t0 = tt * 128
                    ot = out_pool.tile([128, OQ], fp16, tag="out",
                                       name=f"out{q}_{tt}")
                    for j in range(2):
                        ps = psum_pool.tile([128, 512], fp32, tag="mm",
                                            name=f"mm{q}_{tt}_{j}")
                        for k2 in range(K2):
                            nc.tensor.matmul(
                                ps[:],
                                xq_tiles[k2][:, :, t0:t0 + 128],
                                nib_tiles[k2][:, :, j * 512:(j + 1) * 512],
                                start=(k2 == 0), stop=False, perf_mode=DR,
                            )
                        oj = q * 2 + j
                        nc.tensor.matmul(
                            ps[:],
                            eg[:, :, t0:t0 + 128],
                            eh[:, :, oj * 512:(oj + 1) * 512],
                            start=False, stop=True, perf_mode=DR,
                        )
                        nc.vector.tensor_copy(
                            ot[:, j * 512:(j + 1) * 512], ps[:]
                        )
                    nc.sync.dma_start(y[t0:t0 + 128, o0:o0 + OQ], ot[:])
    nc.compile()
    return nc


def _prep_inputs(x, weight_quant, scale, zero, lora_A, lora_B, bias):
    """Host-side layout prep + sharding. Returns in_maps for 8 cores."""
    import ml_dtypes

    f8 = ml_dtypes.float8_e4m3fn
    xf = np.asarray(x, np.float32).reshape(T, I)
    scale = np.asarray(scale, np.float32)
    zero = np.asarray(zero, np.float32)
    lora_A = np.asarray(lora_A, np.float32)
    lora_B = np.asarray(lora_B, np.float32)
    bias = np.asarray(bias, np.float32)

    xs = xf * scale[None, :]
    xq8 = (16.0 * xs).astype(f8)                 # [T, I]

    wq = np.asarray(weight_quant).astype(np.uint8)   # low byte only
    nib = np.empty((O, I), np.float32)
    nib[:, 0::2] = wq & 15
    nib[:, 1::2] = wq >> 4
    nibd8 = ((nib - 7.5) / 16.0).astype(f8)      # [O, I], exact in fp8
    # nibH[k2*128+p, s, o] = nibd8[o, k2*256 + s*128 + p]
    nibH = np.ascontiguousarray(
        nibd8.T.reshape(K2, 2, 128, O).transpose(0, 2, 1, 3).reshape(I // 2, 2, O)
    )

    u = xf @ lora_A.T                            # [T, 8] = x @ A^T
    zc = xs @ (7.5 - zero)                       # [T]
    g9 = (16.0 * zc).astype(f8)
    e1 = g9.astype(np.float32) - 16.0 * zc
    g10 = (-e1).astype(f8)

    Gr = np.zeros((12, T), f8)
    Gr[0:8] = (u.T / 32.0).astype(f8)
    Gr[8] = g9
    Gr[9] = g10
    Gr[10] = np.ones(T, f8)
    Hr = np.zeros((12, O), f8)
    Hr[0:8] = (64.0 * lora_B.T).astype(f8)
    Hr[8] = np.float32(1.0 / 16.0)
    Hr[9] = np.float32(1.0 / 16.0)
    Hr[10] = bias.astype(f8)
    # logical row r -> (p = r//2, s = r%2)
    extH = np.ascontiguousarray(Hr.reshape(6, 2, O))

    in_maps = []
    for c in range(NCORES):
        tsl = slice(c * TC, (c + 1) * TC)
        # xqH[p, k2, s, t] = xq8[t0+t, k2*256 + s*128 + p]
        xqH = np.ascontiguousarray(
            xq8[tsl].reshape(TC, K2, 2, 128).transpose(3, 1, 2, 0)
        )
        extG = np.ascontiguousarray(Gr[:, tsl].reshape(6, 2, TC))
        in_maps.append({
            "xqH": xqH,
            "nibH": nibH,
            "extG": extG,
            "extH": extH,
        })
    return in_maps


def run_on_cores(in_maps, trace=False):
    from concourse.bass_utils import run_bass_kernel_spmd

    if "nc" not in _CACHE:
        _CACHE["nc"] = _build_program()
    return run_bass_kernel_spmd(
        _CACHE["nc"], in_maps, list(range(NCORES)), trace=trace
    )


def kernel(x, weight_quant, scale, zero, lora_A, lora_B, bias):
    x = np.asarray(x)
    weight_quant = np.asarray(weight_quant)

    in_maps = _prep_inputs(x, weight_quant, scale, zero, lora_A, lora_B, bias)
    res = run_on_cores(in_maps).results

    out = np.concatenate(
        [res[c]["y"].astype(np.float32) for c in range(NCORES)], axis=0
    )
    return np.ascontiguousarray(out).reshape(B, S, O)
